# revision 11
# baseline (speedup 1.0000x reference)
"""Trainium2 Bass kernel for nn_AttentionBlock (GroupNorm + single-head
self-attention over 64x64 spatial + out-projection + residual).

Sharding: 8 cores = 4 batches x 2 query-halves. Each core receives its
batch's x as [512, 4096] (channels x pixels), rotated so that its own
2048 query pixels are columns 0:2048. GroupNorm stats / keys / values
span all 4096 pixels (invariant to the rotation), so the program is
identical on every core (pure SPMD, no collectives); the host gathers
the 8 [512, 2048] outputs back into (4, 512, 64, 64).

Algebraic restructuring (host-precomputed):
  - scores^T = h^T (M h + c0),  M = Wk^T Wq, c0 = Wk^T bq
    (the k-bias term is constant within each softmax and cancels).
  - v^T = (Wv h)^T with no bias; bv commutes through the attention
    average and folds into bo2 = out_w @ bv + out_b.
  - softmax without max-subtraction (scores are O(5), exp is safe in
    fp32); denominators applied after the attn@v matmul.
All matmuls run on the PE in fp32r (FP22 multiply, fp32 accumulate).
"""

import numpy as np

import concourse.bass as bass
import concourse.mybir as mybir
import concourse.tile as tile
from concourse.tile_scheduler import N_PROCS
from concourse.vector_clock import ScopedClock, VectorClock

F32 = mybir.dt.float32
F32R = mybir.dt.float32r
AF = mybir.ActivationFunctionType
OP = mybir.AluOpType

PART = 128
C = 512          # channels
N = 4096         # pixels per batch
NQ = 2048        # query pixels per core
CT = C // PART   # 4 channel tiles
NKT = N // PART  # 32 key tiles
CH = 512         # nq chunk width
JCH = NQ // CH   # 4 chunks
EPS = 1e-5
SCALE = float(C) ** -0.5


def _patched_drain_and_barrier(self, tick_clock, wait_clock):
    # Walrus in this container accepts at most one sync-wait per
    # instruction; Tile's stock exit path stacks every outstanding
    # proc's wait on a single SP Drain. Emit one single-wait NOP per
    # proc instead, then a wait-free drain.
    nc = self.nc
    gc = tick_clock.global_clock
    for p in range(N_PROCS):
        t = gc[p]
        if t <= 0:
            continue
        vc = VectorClock([t if q == p else 0 for q in range(N_PROCS)])
        nop = nc.sync.nop(nofuse=True, hint=f"drainwait{p}")
        wait_clock.add_sem_waits(nop.ins, ScopedClock({None: vc}))
    nc.sync.drain()

    nc.all_engine_barrier()
    assert self.sems is not None
    popped = nc._tile_sem_poison_stack.pop()
    assert popped is self._sem_poison
    nc.clear_and_free_semaphores(list(self.sems.allocated().values()))
    nc.all_engine_barrier()


def apply_tile_patch():
    tile.TileContext._drain_and_barrier = _patched_drain_and_barrier


def split_multi_waits(nc):
    """Walrus in this container accepts at most one sync-wait command per
    instruction. Tile's wait-assignment freely stacks several. Hoist all
    but the last wait of each instruction onto single-wait NOPs inserted
    immediately before it on the same engine (engine blocks on each in
    turn, so the gating is equivalent)."""
    k = 0
    for fn in nc.m.functions:
        for bb in fn.blocks:
            il = bb.instructions
            i = 0
            while i < len(il):
                inst = il[i]
                si = inst.sync_info
                waits = list(si.on_wait) if si and si.on_wait else []
                if len(waits) > 1:
                    for w in waits[:-1]:
                        nop = mybir.InstNoOp(name=f"I-waitsplit-{k}")
                        k += 1
                        nop.engine = inst.engine
                        nop.sync_info = mybir.SyncInfo(on_wait=[w], on_update=[])
                        il.insert(i, nop)
                        i += 1
                    si.on_wait = [waits[-1]]
                    inst.sync_info = si
                i += 1


def build_program(split_waits=True):
    apply_tile_patch()
    nc = bass.Bass(name="attnblk")
    xa = nc.dram_tensor("xa", [C, N], F32, kind="ExternalInput").ap()
    mt = nc.dram_tensor("mt", [C, C], F32R, kind="ExternalInput").ap()
    wvt = nc.dram_tensor("wvt", [C, C], F32R, kind="ExternalInput").ap()
    wot = nc.dram_tensor("wot", [C, C], F32R, kind="ExternalInput").ap()
    gw = nc.dram_tensor("gw", [PART, CT], F32, kind="ExternalInput").ap()
    gb = nc.dram_tensor("gb", [PART, CT], F32, kind="ExternalInput").ap()
    c0t = nc.dram_tensor("c0t", [PART, CT], F32, kind="ExternalInput").ap()
    bo2t = nc.dram_tensor("bo2t", [PART, CT], F32, kind="ExternalInput").ap()
    gmat = nc.dram_tensor("gmat", [PART, 8], F32R, kind="ExternalInput").ap()
    gmatt = nc.dram_tensor("gmatt", [8, PART], F32R, kind="ExternalInput").ap()
    onesd = nc.dram_tensor("onesd", [PART, PART], F32R, kind="ExternalInput").ap()
    y = nc.dram_tensor("y", [C, NQ], F32, kind="ExternalOutput").ap()

    with tile.TileContext(nc) as tc:
        with (
            tc.tile_pool(name="const", bufs=1) as cp,
            tc.tile_pool(name="wts", bufs=1) as wp,
            tc.tile_pool(name="hp", bufs=1) as hp,
        ):
            gwt = cp.tile([PART, CT], F32)
            nc.sync.dma_start(out=gwt, in_=gw)
            gbt = cp.tile([PART, CT], F32)
            nc.sync.dma_start(out=gbt, in_=gb)
            c0s = cp.tile([PART, CT], F32)
            nc.sync.dma_start(out=c0s, in_=c0t)
            bo2s = cp.tile([PART, CT], F32)
            nc.sync.dma_start(out=bo2s, in_=bo2t)
            gm = cp.tile([PART, 8], F32R)
            nc.sync.dma_start(out=gm, in_=gmat)
            gmt = cp.tile([8, PART], F32R)
            nc.sync.dma_start(out=gmt, in_=gmatt)
            ones = cp.tile([PART, PART], F32R)
            nc.sync.dma_start(out=ones, in_=onesd)
            epst = cp.tile([PART, 1], F32)
            nc.vector.memset(epst, EPS)

            mts = wp.tile([PART, CT, C], F32R)
            wvts = wp.tile([PART, CT, C], F32R)
            wots = wp.tile([PART, CT, C], F32R)
            for j in range(CT):
                nc.sync.dma_start(out=mts[:, j, :], in_=mt[j * PART:(j + 1) * PART, :])
                nc.sync.dma_start(out=wvts[:, j, :], in_=wvt[j * PART:(j + 1) * PART, :])
                nc.sync.dma_start(out=wots[:, j, :], in_=wot[j * PART:(j + 1) * PART, :])

            hts = [hp.tile([PART, N], F32R, tag=f"h{ci}", name=f"h{ci}") for ci in range(CT)]

            # ---- Phase A+B: GroupNorm stats + normalize into h ----
            with (
                tc.tile_pool(name="stats", bufs=2) as sp,
                tc.tile_pool(name="coef", bufs=1) as cfp,
                tc.tile_pool(name="xst", bufs=2) as xp,
                tc.tile_pool(name="pst", bufs=2, space="PSUM") as pp,
            ):
                acoef = cfp.tile([PART, CT], F32)
                bcoef = cfp.tile([PART, CT], F32)
                for ci in range(CT):
                    xt = xp.tile([PART, N], F32, tag="x")
                    nc.sync.dma_start(out=xt, in_=xa[ci * PART:(ci + 1) * PART, :])
                    xr = xt.rearrange("p (s f) -> p s f", f=512)
                    st6 = sp.tile([PART, 8, 6], F32, tag="st6")
                    for s in range(8):
                        nc.vector.bn_stats(out=st6[:, s, :], in_=xr[:, s, :])
                    mv = sp.tile([PART, 2], F32, tag="mv")
                    nc.vector.bn_aggr(out=mv, in_=st6)
                    # s2 = (mean, E[x^2]) per channel
                    s2 = sp.tile([PART, 2], F32R, tag="s2")
                    nc.vector.tensor_copy(out=s2[:, 0:1], in_=mv[:, 0:1])
                    nc.vector.tensor_tensor(
                        out=s2[:, 1:2], in0=mv[:, 0:1], in1=mv[:, 0:1], op=OP.mult)
                    nc.vector.tensor_add(out=s2[:, 1:2], in0=s2[:, 1:2], in1=mv[:, 1:2])
                    # group means over 16-channel blocks: [8, 2]
                    gp_ = pp.tile([8, 2], F32, tag="gp")
                    nc.tensor.matmul(gp_, lhsT=gm, rhs=s2,
                                     start=True, stop=True)
                    gs = sp.tile([8, 2], F32R, tag="gs")
                    nc.vector.tensor_copy(out=gs, in_=gp_)
                    msq = sp.tile([8, 1], F32, tag="msq")
                    nc.vector.tensor_tensor(
                        out=msq, in0=gs[:, 0:1], in1=gs[:, 0:1], op=OP.mult)
                    nc.vector.tensor_sub(out=gs[:, 1:2], in0=gs[:, 1:2], in1=msq)
                    nc.scalar.activation(out=gs[:, 1:2], in_=gs[:, 1:2],
                                         func=AF.Sqrt, bias=epst[0:8])
                    with nc.allow_low_precision(
                            reason="fp32r rounding for PE broadcast matmul"):
                        nc.vector.reciprocal(out=gs[:, 1:2], in_=gs[:, 1:2])
                    # broadcast per-group (mean, rstd) back to channels
                    cb = pp.tile([PART, 2], F32, tag="cb")
                    nc.tensor.matmul(cb, lhsT=gmt, rhs=gs,
                                     start=True, stop=True)
                    nc.vector.tensor_tensor(
                        out=acoef[:, ci:ci + 1], in0=cb[:, 1:2],
                        in1=gwt[:, ci:ci + 1], op=OP.mult)
                    tmpb = sp.tile([PART, 1], F32, tag="tmpb")
                    nc.vector.tensor_tensor(
                        out=tmpb, in0=cb[:, 0:1], in1=acoef[:, ci:ci + 1], op=OP.mult)
                    nc.vector.tensor_sub(
                        out=bcoef[:, ci:ci + 1], in0=gbt[:, ci:ci + 1], in1=tmpb)
                    # h = x * a + b
                    nc.vector.tensor_scalar(
                        out=hts[ci], in0=xt,
                        scalar1=acoef[:, ci:ci + 1], scalar2=bcoef[:, ci:ci + 1],
                        op0=OP.mult, op1=OP.add)

            # ---- Phase C: v^T tiles ----
            with tc.tile_pool(name="vtp", bufs=1) as vp:
                vts = vp.tile([PART, NKT, CH], F32R)
                with tc.tile_pool(name="vps", bufs=2, space="PSUM") as vpp:
                    for t in range(NKT):
                        vps = vpp.tile([PART, CH], F32, tag="vps")
                        for ci in range(CT):
                            nc.tensor.matmul(
                                vps,
                                lhsT=hts[ci][:, t * PART:(t + 1) * PART],
                                rhs=wvts[:, ci, :],
                                start=(ci == 0), stop=(ci == CT - 1))
                        nc.vector.tensor_copy(out=vts[:, t, :], in_=vps)

                # ---- Phase D+E: attention + out-projection, per nq-chunk ----
                with (
                    tc.tile_pool(name="ujp", bufs=1) as up,
                    tc.tile_pool(name="ep", bufs=3) as ep,
                    tc.tile_pool(name="attp", bufs=1) as ap_,
                    tc.tile_pool(name="rcp", bufs=2) as rp,
                    tc.tile_pool(name="xrp", bufs=2) as xrp,
                    tc.tile_pool(name="otp", bufs=2) as otp,
                    tc.tile_pool(name="oup", bufs=1, space="PSUM") as oup,
                    tc.tile_pool(name="stp", bufs=2, space="PSUM") as stp,
                    tc.tile_pool(name="ddp", bufs=1, space="PSUM") as ddp,
                    tc.tile_pool(name="upp", bufs=1, space="PSUM") as upp,
                ):
                    for j in range(JCH):
                        jsl = slice(j * CH, (j + 1) * CH)
                        # u_j = M h[:, jsl] + c0
                        uj = up.tile([PART, CT, CH], F32R, tag="uj")
                        for i in range(CT):
                            ups = upp.tile([PART, CH], F32, tag="up")
                            for jj in range(CT):
                                nc.tensor.matmul(
                                    ups,
                                    lhsT=mts[:, jj, i * PART:(i + 1) * PART],
                                    rhs=hts[jj][:, jsl],
                                    start=(jj == 0), stop=(jj == CT - 1))
                            nc.vector.tensor_scalar(
                                out=uj[:, i, :], in0=ups,
                                scalar1=c0s[:, i:i + 1], scalar2=None, op0=OP.add)
                        # attention accumulation over key tiles
                        ou = [oup.tile([PART, CH], F32, tag=f"ou{m}", name=f"ou{m}_{j}") for m in range(CT)]
                        dd = ddp.tile([PART, CH], F32, tag="dd")
                        for t in range(NKT):
                            st = stp.tile([PART, CH], F32, tag="st")
                            for ci in range(CT):
                                nc.tensor.matmul(
                                    st,
                                    lhsT=hts[ci][:, t * PART:(t + 1) * PART],
                                    rhs=uj[:, ci, :],
                                    start=(ci == 0), stop=(ci == CT - 1))
                            et = ep.tile([PART, CH], F32R, tag="et")
                            nc.scalar.activation(out=et, in_=st, func=AF.Exp, scale=SCALE)
                            etr = et[:]
                            for m in range(CT):
                                nc.tensor.matmul(
                                    ou[m],
                                    lhsT=vts[:, t, m * PART:(m + 1) * PART],
                                    rhs=etr,
                                    start=(t == 0), stop=(t == NKT - 1))
                            nc.tensor.matmul(
                                dd, lhsT=ones, rhs=etr,
                                start=(t == 0), stop=(t == NKT - 1))
                        # normalize
                        rc = rp.tile([PART, CH], F32, tag="rc")
                        nc.vector.reciprocal(out=rc, in_=dd)
                        att = ap_.tile([PART, CT, CH], F32R, tag="att")
                        for m in range(CT):
                            nc.vector.tensor_mul(out=att[:, m, :], in0=ou[m], in1=rc)
                        # out-projection + bias + residual
                        for m in range(CT):
                            fp = oup.tile([PART, CH], F32, tag=f"ou{m}")
                            for ci in range(CT):
                                nc.tensor.matmul(
                                    fp,
                                    lhsT=wots[:, ci, m * PART:(m + 1) * PART],
                                    rhs=att[:, ci, :],
                                    start=(ci == 0), stop=(ci == CT - 1))
                            xr_ = xrp.tile([PART, CH], F32, tag="xr")
                            nc.sync.dma_start(
                                out=xr_, in_=xa[m * PART:(m + 1) * PART, jsl])
                            ot = otp.tile([PART, CH], F32, tag="ot")
                            nc.vector.tensor_scalar(
                                out=ot, in0=fp,
                                scalar1=bo2s[:, m:m + 1], scalar2=None, op0=OP.add)
                            nc.vector.tensor_add(out=ot, in0=ot, in1=xr_)
                            nc.sync.dma_start(
                                out=y[m * PART:(m + 1) * PART, jsl], in_=ot)
    if split_waits:
        split_multi_waits(nc)
    return nc


def prep_inputs(x, gn_w, gn_b, qkv_w, qkv_b, out_w, out_b):
    x = np.asarray(x, np.float32)
    gn_w = np.asarray(gn_w, np.float32)
    gn_b = np.asarray(gn_b, np.float32)
    qkv_w = np.asarray(qkv_w, np.float32)
    qkv_b = np.asarray(qkv_b, np.float32)
    out_w = np.asarray(out_w, np.float32)
    out_b = np.asarray(out_b, np.float32)

    Wq, Wk, Wv = qkv_w[0:C], qkv_w[C:2 * C], qkv_w[2 * C:3 * C]
    bq, bv = qkv_b[0:C], qkv_b[2 * C:3 * C]
    mt = np.ascontiguousarray(Wq.T @ Wk, dtype=np.float32)       # (M = Wk^T Wq).T
    wvt = np.ascontiguousarray(Wv.T, dtype=np.float32)
    wot = np.ascontiguousarray(out_w.T, dtype=np.float32)
    c0 = (Wk.T @ bq).astype(np.float32)
    bo2 = (out_w @ bv + out_b).astype(np.float32)

    def coltiles(v):
        return np.ascontiguousarray(v.reshape(CT, PART).T, dtype=np.float32)

    gmat = np.zeros((PART, 8), np.float32)
    gmatt = np.zeros((8, PART), np.float32)
    for p in range(PART):
        gmat[p, p // 16] = 1.0 / 16.0
        gmatt[p // 16, p] = 1.0
    shared = {
        "mt": mt, "wvt": wvt, "wot": wot,
        "gw": coltiles(gn_w), "gb": coltiles(gn_b),
        "c0t": coltiles(c0), "bo2t": coltiles(bo2),
        "gmat": gmat, "gmatt": gmatt,
        "onesd": np.ones((PART, PART), np.float32),
    }
    in_maps = []
    for core in range(8):
        br, hf = divmod(core, 2)
        xa = x[br].reshape(C, N)
        if hf:
            xa = np.concatenate([xa[:, NQ:], xa[:, :NQ]], axis=1)
        xa = np.ascontiguousarray(xa, dtype=np.float32)
        in_maps.append({"xa": xa, **shared})
    return in_maps


def assemble_output(results, b=4, hh=64, ww=64):
    out = np.zeros((b, C, N), np.float32)
    for core in range(8):
        br, hf = divmod(core, 2)
        out[br][:, hf * NQ:(hf + 1) * NQ] = results[core]["y"]
    return out.reshape(b, C, hh, ww)


def kernel(x, gn_w, gn_b, qkv_w, qkv_b, out_w, out_b):
    from concourse import bass_utils
    in_maps = prep_inputs(x, gn_w, gn_b, qkv_w, qkv_b, out_w, out_b)
    nc = build_program()
    res = bass_utils.run_bass_kernel_spmd(nc, in_maps, core_ids=list(range(8)))
    return assemble_output(res.results)


# revision 15
# speedup vs baseline: 1.0907x; 1.0907x over previous
"""Trainium2 Bass kernel for nn_AttentionBlock (GroupNorm + single-head
self-attention over 64x64 spatial + out-projection + residual).

Sharding: 8 cores = 4 batches x 2 query-halves. Each core receives its
batch's x as [512, 4096] (channels x pixels), rotated so that its own
2048 query pixels are columns 0:2048. GroupNorm stats / keys / values
span all 4096 pixels (invariant to the rotation), so the program is
identical on every core (pure SPMD, no collectives); the host gathers
the 8 [512, 2048] outputs back into (4, 512, 64, 64).

Algebraic restructuring (host-precomputed):
  - scores^T = h^T (M h + c0),  M = Wk^T Wq, c0 = Wk^T bq
    (the k-bias term is constant within each softmax and cancels).
  - v^T = (Wv h)^T with no bias; bv commutes through the attention
    average and folds into bo2 = out_w @ bv + out_b.
  - softmax without max-subtraction (scores are O(5), exp is safe in
    fp32); denominators applied after the attn@v matmul.
All matmuls run on the PE in fp32r (FP22 multiply, fp32 accumulate).
"""

import numpy as np
import ml_dtypes

import concourse.bass as bass
import concourse.mybir as mybir
import concourse.tile as tile
from concourse.tile_scheduler import N_PROCS
from concourse.vector_clock import ScopedClock, VectorClock

F32 = mybir.dt.float32
F32R = mybir.dt.float32r
BT = mybir.dt.bfloat16
AF = mybir.ActivationFunctionType
OP = mybir.AluOpType

PART = 128
C = 512          # channels
N = 4096         # pixels per batch
NQ = 2048        # query pixels per core
CT = C // PART   # 4 channel tiles
NKT = N // PART  # 32 key tiles
CH = 512         # nq chunk width
JCH = NQ // CH   # 4 chunks
EPS = 1e-5
SCALE = float(C) ** -0.5


def _patched_drain_and_barrier(self, tick_clock, wait_clock):
    # Walrus in this container accepts at most one sync-wait per
    # instruction; Tile's stock exit path stacks every outstanding
    # proc's wait on a single SP Drain. Emit one single-wait NOP per
    # proc instead, then a wait-free drain.
    nc = self.nc
    gc = tick_clock.global_clock
    for p in range(N_PROCS):
        t = gc[p]
        if t <= 0:
            continue
        vc = VectorClock([t if q == p else 0 for q in range(N_PROCS)])
        nop = nc.sync.nop(nofuse=True, hint=f"drainwait{p}")
        wait_clock.add_sem_waits(nop.ins, ScopedClock({None: vc}))
    nc.sync.drain()

    nc.all_engine_barrier()
    assert self.sems is not None
    popped = nc._tile_sem_poison_stack.pop()
    assert popped is self._sem_poison
    nc.clear_and_free_semaphores(list(self.sems.allocated().values()))
    nc.all_engine_barrier()


def apply_tile_patch():
    tile.TileContext._drain_and_barrier = _patched_drain_and_barrier


def split_multi_waits(nc):
    """Walrus in this container accepts at most one sync-wait command per
    instruction. Tile's wait-assignment freely stacks several. Hoist all
    but the last wait of each instruction onto single-wait NOPs inserted
    immediately before it on the same engine (engine blocks on each in
    turn, so the gating is equivalent)."""
    k = 0
    for fn in nc.m.functions:
        for bb in fn.blocks:
            il = bb.instructions
            i = 0
            while i < len(il):
                inst = il[i]
                si = inst.sync_info
                waits = list(si.on_wait) if si and si.on_wait else []
                if len(waits) > 1:
                    for w in waits[:-1]:
                        nop = mybir.InstNoOp(name=f"I-waitsplit-{k}")
                        k += 1
                        nop.engine = inst.engine
                        nop.sync_info = mybir.SyncInfo(on_wait=[w], on_update=[])
                        il.insert(i, nop)
                        i += 1
                    si.on_wait = [waits[-1]]
                    inst.sync_info = si
                i += 1


def build_program(split_waits=True):
    apply_tile_patch()
    nc = bass.Bass(name="attnblk")
    xa = nc.dram_tensor("xa", [C, N], F32, kind="ExternalInput").ap()
    mt = nc.dram_tensor("mt", [C, C], BT, kind="ExternalInput").ap()
    wvt = nc.dram_tensor("wvt", [C, C], BT, kind="ExternalInput").ap()
    wot = nc.dram_tensor("wot", [C, C], BT, kind="ExternalInput").ap()
    gw = nc.dram_tensor("gw", [PART, CT], F32, kind="ExternalInput").ap()
    gb = nc.dram_tensor("gb", [PART, CT], F32, kind="ExternalInput").ap()
    c0t = nc.dram_tensor("c0t", [PART, CT], F32, kind="ExternalInput").ap()
    bo2t = nc.dram_tensor("bo2t", [PART, CT], F32, kind="ExternalInput").ap()
    gmat = nc.dram_tensor("gmat", [PART, 8], F32R, kind="ExternalInput").ap()
    gmatt = nc.dram_tensor("gmatt", [8, PART], F32R, kind="ExternalInput").ap()
    onesd = nc.dram_tensor("onesd", [PART, PART], BT, kind="ExternalInput").ap()
    y = nc.dram_tensor("y", [C, NQ], F32, kind="ExternalOutput").ap()

    with tile.TileContext(nc) as tc:
        with (
            tc.tile_pool(name="const", bufs=1) as cp,
            tc.tile_pool(name="wts", bufs=1) as wp,
            tc.tile_pool(name="hp", bufs=1) as hp,
        ):
            gwt = cp.tile([PART, CT], F32)
            nc.sync.dma_start(out=gwt, in_=gw)
            gbt = cp.tile([PART, CT], F32)
            nc.sync.dma_start(out=gbt, in_=gb)
            c0s = cp.tile([PART, CT], F32)
            nc.sync.dma_start(out=c0s, in_=c0t)
            bo2s = cp.tile([PART, CT], F32)
            nc.sync.dma_start(out=bo2s, in_=bo2t)
            gm = cp.tile([PART, 8], F32R)
            nc.sync.dma_start(out=gm, in_=gmat)
            gmt = cp.tile([8, PART], F32R)
            nc.sync.dma_start(out=gmt, in_=gmatt)
            ones = cp.tile([PART, PART], BT)
            nc.sync.dma_start(out=ones, in_=onesd)
            epst = cp.tile([PART, 1], F32)
            nc.vector.memset(epst, EPS)

            mts = wp.tile([PART, CT, C], BT)
            wvts = wp.tile([PART, CT, C], BT)
            wots = wp.tile([PART, CT, C], BT)
            for j in range(CT):
                nc.sync.dma_start(out=mts[:, j, :], in_=mt[j * PART:(j + 1) * PART, :])
                nc.sync.dma_start(out=wvts[:, j, :], in_=wvt[j * PART:(j + 1) * PART, :])
                nc.sync.dma_start(out=wots[:, j, :], in_=wot[j * PART:(j + 1) * PART, :])

            hts = [hp.tile([PART, N], BT, tag=f"h{ci}", name=f"h{ci}") for ci in range(CT)]

            # ---- Phase A+B: GroupNorm stats + normalize into h ----
            with (
                tc.tile_pool(name="stats", bufs=2) as sp,
                tc.tile_pool(name="coef", bufs=1) as cfp,
                tc.tile_pool(name="xst", bufs=4) as xp,
                tc.tile_pool(name="pst", bufs=2, space="PSUM") as pp,
            ):
                acoef = cfp.tile([PART, CT], F32)
                bcoef = cfp.tile([PART, CT], F32)
                for ci in range(CT):
                    xt = xp.tile([PART, N], F32, tag="x")
                    nc.sync.dma_start(out=xt, in_=xa[ci * PART:(ci + 1) * PART, :])
                    xr = xt.rearrange("p (s f) -> p s f", f=512)
                    st6 = sp.tile([PART, 8, 6], F32, tag="st6")
                    for s in range(8):
                        nc.vector.bn_stats(out=st6[:, s, :], in_=xr[:, s, :])
                    mv = sp.tile([PART, 2], F32, tag="mv")
                    nc.vector.bn_aggr(out=mv, in_=st6)
                    # s2 = (mean, E[x^2]) per channel
                    s2 = sp.tile([PART, 2], F32R, tag="s2")
                    nc.vector.tensor_copy(out=s2[:, 0:1], in_=mv[:, 0:1])
                    nc.vector.tensor_tensor(
                        out=s2[:, 1:2], in0=mv[:, 0:1], in1=mv[:, 0:1], op=OP.mult)
                    nc.vector.tensor_add(out=s2[:, 1:2], in0=s2[:, 1:2], in1=mv[:, 1:2])
                    # group means over 16-channel blocks: [8, 2]
                    gp_ = pp.tile([8, 2], F32, tag="gp")
                    nc.tensor.matmul(gp_, lhsT=gm, rhs=s2,
                                     start=True, stop=True)
                    gs = sp.tile([8, 2], F32R, tag="gs")
                    nc.vector.tensor_copy(out=gs, in_=gp_)
                    msq = sp.tile([8, 1], F32, tag="msq")
                    nc.vector.tensor_tensor(
                        out=msq, in0=gs[:, 0:1], in1=gs[:, 0:1], op=OP.mult)
                    nc.vector.tensor_sub(out=gs[:, 1:2], in0=gs[:, 1:2], in1=msq)
                    nc.scalar.activation(out=gs[:, 1:2], in_=gs[:, 1:2],
                                         func=AF.Sqrt, bias=epst[0:8])
                    with nc.allow_low_precision(
                            reason="fp32r rounding for PE broadcast matmul"):
                        nc.vector.reciprocal(out=gs[:, 1:2], in_=gs[:, 1:2])
                    # broadcast per-group (mean, rstd) back to channels
                    cb = pp.tile([PART, 2], F32, tag="cb")
                    nc.tensor.matmul(cb, lhsT=gmt, rhs=gs,
                                     start=True, stop=True)
                    nc.vector.tensor_tensor(
                        out=acoef[:, ci:ci + 1], in0=cb[:, 1:2],
                        in1=gwt[:, ci:ci + 1], op=OP.mult)
                    tmpb = sp.tile([PART, 1], F32, tag="tmpb")
                    nc.vector.tensor_tensor(
                        out=tmpb, in0=cb[:, 0:1], in1=acoef[:, ci:ci + 1], op=OP.mult)
                    nc.vector.tensor_sub(
                        out=bcoef[:, ci:ci + 1], in0=gbt[:, ci:ci + 1], in1=tmpb)
                    # h = x * a + b
                    nc.vector.tensor_scalar(
                        out=hts[ci], in0=xt,
                        scalar1=acoef[:, ci:ci + 1], scalar2=bcoef[:, ci:ci + 1],
                        op0=OP.mult, op1=OP.add)

            # ---- Phase C: v^T tiles ----
            with tc.tile_pool(name="vtp", bufs=1) as vp:
                vts = vp.tile([PART, NKT, CH], BT)
                with tc.tile_pool(name="vps", bufs=2, space="PSUM") as vpp:
                    for t in range(NKT):
                        vps = vpp.tile([PART, CH], F32, tag="vps")
                        for ci in range(CT):
                            nc.tensor.matmul(
                                vps,
                                lhsT=hts[ci][:, t * PART:(t + 1) * PART],
                                rhs=wvts[:, ci, :],
                                start=(ci == 0), stop=(ci == CT - 1))
                        nc.vector.tensor_copy(out=vts[:, t, :], in_=vps)

                # ---- Phase D+E: attention + out-projection, per nq-chunk ----
                with (
                    tc.tile_pool(name="ujp", bufs=2) as up,
                    tc.tile_pool(name="ep", bufs=3) as ep,
                    tc.tile_pool(name="attp", bufs=1) as ap_,
                    tc.tile_pool(name="rcp", bufs=2) as rp,
                    tc.tile_pool(name="xrp", bufs=2) as xrp,
                    tc.tile_pool(name="otp", bufs=2) as otp,
                    tc.tile_pool(name="oup", bufs=1, space="PSUM") as oup,
                    tc.tile_pool(name="stp", bufs=2, space="PSUM") as stp,
                    tc.tile_pool(name="ddp", bufs=1, space="PSUM") as ddp,
                    tc.tile_pool(name="upp", bufs=1, space="PSUM") as upp,
                ):
                    def compute_u(jc):
                        # u_jc = M h[:, chunk jc] + c0
                        ut = up.tile([PART, CT, CH], BT, tag="uj", name=f"uj{jc}")
                        sl = slice(jc * CH, (jc + 1) * CH)
                        for i in range(CT):
                            ups = upp.tile([PART, CH], F32, tag="up")
                            for jj in range(CT):
                                nc.tensor.matmul(
                                    ups,
                                    lhsT=mts[:, jj, i * PART:(i + 1) * PART],
                                    rhs=hts[jj][:, sl],
                                    start=(jj == 0), stop=(jj == CT - 1))
                            nc.vector.tensor_scalar(
                                out=ut[:, i, :], in0=ups,
                                scalar1=c0s[:, i:i + 1], scalar2=None, op0=OP.add)
                        return ut

                    uj_next = compute_u(0)
                    for j in range(JCH):
                        jsl = slice(j * CH, (j + 1) * CH)
                        uj = uj_next
                        # attention accumulation over key tiles
                        ou = [oup.tile([PART, CH], F32, tag=f"ou{m}", name=f"ou{m}_{j}") for m in range(CT)]
                        dd = ddp.tile([PART, CH], F32, tag="dd")
                        for t in range(NKT):
                            st = stp.tile([PART, CH], F32, tag="st")
                            for ci in range(CT):
                                nc.tensor.matmul(
                                    st,
                                    lhsT=hts[ci][:, t * PART:(t + 1) * PART],
                                    rhs=uj[:, ci, :],
                                    start=(ci == 0), stop=(ci == CT - 1))
                            et = ep.tile([PART, CH], BT, tag="et")
                            nc.scalar.activation(out=et, in_=st, func=AF.Exp, scale=SCALE)
                            etr = et[:]
                            for m in range(CT):
                                nc.tensor.matmul(
                                    ou[m],
                                    lhsT=vts[:, t, m * PART:(m + 1) * PART],
                                    rhs=etr,
                                    start=(t == 0), stop=(t == NKT - 1))
                            nc.tensor.matmul(
                                dd, lhsT=ones, rhs=etr,
                                start=(t == 0), stop=(t == NKT - 1))
                            if t == NKT - 3 and j + 1 < JCH:
                                uj_next = compute_u(j + 1)
                        # normalize
                        rc = rp.tile([PART, CH], F32, tag="rc")
                        nc.vector.reciprocal(out=rc, in_=dd)
                        att = ap_.tile([PART, CT, CH], BT, tag="att")
                        for m in range(CT):
                            nc.vector.tensor_mul(out=att[:, m, :], in0=ou[m], in1=rc)
                        # out-projection + bias + residual
                        for m in range(CT):
                            fp = oup.tile([PART, CH], F32, tag=f"ou{m}")
                            for ci in range(CT):
                                nc.tensor.matmul(
                                    fp,
                                    lhsT=wots[:, ci, m * PART:(m + 1) * PART],
                                    rhs=att[:, ci, :],
                                    start=(ci == 0), stop=(ci == CT - 1))
                            xr_ = xrp.tile([PART, CH], F32, tag="xr")
                            nc.sync.dma_start(
                                out=xr_, in_=xa[m * PART:(m + 1) * PART, jsl])
                            ot = otp.tile([PART, CH], F32, tag="ot")
                            nc.vector.tensor_scalar(
                                out=ot, in0=fp,
                                scalar1=bo2s[:, m:m + 1], scalar2=None, op0=OP.add)
                            nc.vector.tensor_add(out=ot, in0=ot, in1=xr_)
                            nc.sync.dma_start(
                                out=y[m * PART:(m + 1) * PART, jsl], in_=ot)
    if split_waits:
        split_multi_waits(nc)
    return nc


def prep_inputs(x, gn_w, gn_b, qkv_w, qkv_b, out_w, out_b):
    x = np.asarray(x, np.float32)
    gn_w = np.asarray(gn_w, np.float32)
    gn_b = np.asarray(gn_b, np.float32)
    qkv_w = np.asarray(qkv_w, np.float32)
    qkv_b = np.asarray(qkv_b, np.float32)
    out_w = np.asarray(out_w, np.float32)
    out_b = np.asarray(out_b, np.float32)

    Wq, Wk, Wv = qkv_w[0:C], qkv_w[C:2 * C], qkv_w[2 * C:3 * C]
    bq, bv = qkv_b[0:C], qkv_b[2 * C:3 * C]
    bf16 = ml_dtypes.bfloat16
    mt = np.ascontiguousarray((Wq.T @ Wk).astype(bf16))          # (M = Wk^T Wq).T
    wvt = np.ascontiguousarray(Wv.T.astype(bf16))
    wot = np.ascontiguousarray(out_w.T.astype(bf16))
    c0 = (Wk.T @ bq).astype(np.float32)
    bo2 = (out_w @ bv + out_b).astype(np.float32)

    def coltiles(v):
        return np.ascontiguousarray(v.reshape(CT, PART).T, dtype=np.float32)

    gmat = np.zeros((PART, 8), np.float32)
    gmatt = np.zeros((8, PART), np.float32)
    for p in range(PART):
        gmat[p, p // 16] = 1.0 / 16.0
        gmatt[p // 16, p] = 1.0
    shared = {
        "mt": mt, "wvt": wvt, "wot": wot,
        "gw": coltiles(gn_w), "gb": coltiles(gn_b),
        "c0t": coltiles(c0), "bo2t": coltiles(bo2),
        "gmat": gmat, "gmatt": gmatt,
        "onesd": np.ones((PART, PART), ml_dtypes.bfloat16),
    }
    in_maps = []
    for core in range(8):
        br, hf = divmod(core, 2)
        xa = x[br].reshape(C, N)
        if hf:
            xa = np.concatenate([xa[:, NQ:], xa[:, :NQ]], axis=1)
        xa = np.ascontiguousarray(xa, dtype=np.float32)
        in_maps.append({"xa": xa, **shared})
    return in_maps


def assemble_output(results, b=4, hh=64, ww=64):
    out = np.zeros((b, C, N), np.float32)
    for core in range(8):
        br, hf = divmod(core, 2)
        out[br][:, hf * NQ:(hf + 1) * NQ] = results[core]["y"]
    return out.reshape(b, C, hh, ww)


def kernel(x, gn_w, gn_b, qkv_w, qkv_b, out_w, out_b):
    from concourse import bass_utils
    in_maps = prep_inputs(x, gn_w, gn_b, qkv_w, qkv_b, out_w, out_b)
    nc = build_program()
    res = bass_utils.run_bass_kernel_spmd(nc, in_maps, core_ids=list(range(8)))
    return assemble_output(res.results)


# revision 16
# speedup vs baseline: 1.2653x; 1.1601x over previous
"""Trainium2 Bass kernel for nn_AttentionBlock (GroupNorm + single-head
self-attention over 64x64 spatial + out-projection + residual).

Sharding: 8 cores = 4 batches x 2 query-halves. Each core receives its
batch's x as [512, 4096] (channels x pixels), rotated so that its own
2048 query pixels are columns 0:2048. GroupNorm stats / keys / values
span all 4096 pixels (invariant to the rotation), so the program is
identical on every core (pure SPMD, no collectives); the host gathers
the 8 [512, 2048] outputs back into (4, 512, 64, 64).

Algebraic restructuring (host-precomputed):
  - scores^T = h^T (M h + c0),  M = Wk^T Wq, c0 = Wk^T bq
    (the k-bias term is constant within each softmax and cancels).
  - v^T = (Wv h)^T with no bias; bv commutes through the attention
    average and folds into bo2 = out_w @ bv + out_b.
  - softmax without max-subtraction (scores are O(5), exp is safe in
    fp32); denominators applied after the attn@v matmul.
All matmuls run on the PE in fp32r (FP22 multiply, fp32 accumulate).
"""

import numpy as np
import ml_dtypes

import concourse.bass as bass
import concourse.mybir as mybir
import concourse.tile as tile
from concourse.tile_scheduler import N_PROCS
from concourse.vector_clock import ScopedClock, VectorClock

F32 = mybir.dt.float32
F32R = mybir.dt.float32r
BT = mybir.dt.bfloat16
AF = mybir.ActivationFunctionType
OP = mybir.AluOpType

PART = 128
C = 512          # channels
N = 4096         # pixels per batch
NQ = 2048        # query pixels per core
CT = C // PART   # 4 channel tiles
NKT = N // PART  # 32 key tiles
CH = 512         # nq chunk width
JCH = NQ // CH   # 4 chunks
EPS = 1e-5
SCALE = float(C) ** -0.5


def _patched_drain_and_barrier(self, tick_clock, wait_clock):
    # Walrus in this container accepts at most one sync-wait per
    # instruction; Tile's stock exit path stacks every outstanding
    # proc's wait on a single SP Drain. Emit one single-wait NOP per
    # proc instead, then a wait-free drain.
    nc = self.nc
    gc = tick_clock.global_clock
    for p in range(N_PROCS):
        t = gc[p]
        if t <= 0:
            continue
        vc = VectorClock([t if q == p else 0 for q in range(N_PROCS)])
        nop = nc.sync.nop(nofuse=True, hint=f"drainwait{p}")
        wait_clock.add_sem_waits(nop.ins, ScopedClock({None: vc}))
    nc.sync.drain()

    nc.all_engine_barrier()
    assert self.sems is not None
    popped = nc._tile_sem_poison_stack.pop()
    assert popped is self._sem_poison
    nc.clear_and_free_semaphores(list(self.sems.allocated().values()))
    nc.all_engine_barrier()


def apply_tile_patch():
    tile.TileContext._drain_and_barrier = _patched_drain_and_barrier


def split_multi_waits(nc):
    """Walrus in this container accepts at most one sync-wait command per
    instruction. Tile's wait-assignment freely stacks several. Hoist all
    but the last wait of each instruction onto single-wait NOPs inserted
    immediately before it on the same engine (engine blocks on each in
    turn, so the gating is equivalent)."""
    k = 0
    for fn in nc.m.functions:
        for bb in fn.blocks:
            il = bb.instructions
            i = 0
            while i < len(il):
                inst = il[i]
                si = inst.sync_info
                waits = list(si.on_wait) if si and si.on_wait else []
                if len(waits) > 1:
                    for w in waits[:-1]:
                        nop = mybir.InstNoOp(name=f"I-waitsplit-{k}")
                        k += 1
                        nop.engine = inst.engine
                        nop.sync_info = mybir.SyncInfo(on_wait=[w], on_update=[])
                        il.insert(i, nop)
                        i += 1
                    si.on_wait = [waits[-1]]
                    inst.sync_info = si
                i += 1


def build_program(split_waits=True):
    apply_tile_patch()
    nc = bass.Bass(name="attnblk")
    xa = nc.dram_tensor("xa", [C, N], F32, kind="ExternalInput").ap()
    mt = nc.dram_tensor("mt", [C, C], BT, kind="ExternalInput").ap()
    wvt = nc.dram_tensor("wvt", [C, C], BT, kind="ExternalInput").ap()
    wot = nc.dram_tensor("wot", [C, C], BT, kind="ExternalInput").ap()
    gw = nc.dram_tensor("gw", [PART, CT], F32, kind="ExternalInput").ap()
    gb = nc.dram_tensor("gb", [PART, CT], F32, kind="ExternalInput").ap()
    c0t = nc.dram_tensor("c0t", [PART, CT], F32, kind="ExternalInput").ap()
    bo2t = nc.dram_tensor("bo2t", [PART, CT], F32, kind="ExternalInput").ap()
    gmat = nc.dram_tensor("gmat", [PART, 8], F32R, kind="ExternalInput").ap()
    gmatt = nc.dram_tensor("gmatt", [8, PART], F32R, kind="ExternalInput").ap()
    onesd = nc.dram_tensor("onesd", [PART, PART], BT, kind="ExternalInput").ap()
    y = nc.dram_tensor("y", [C, NQ], F32, kind="ExternalOutput").ap()

    with tile.TileContext(nc) as tc:
        with (
            tc.tile_pool(name="const", bufs=1) as cp,
            tc.tile_pool(name="wts", bufs=1) as wp,
            tc.tile_pool(name="hp", bufs=1) as hp,
        ):
            gwt = cp.tile([PART, CT], F32)
            nc.sync.dma_start(out=gwt, in_=gw)
            gbt = cp.tile([PART, CT], F32)
            nc.sync.dma_start(out=gbt, in_=gb)
            c0s = cp.tile([PART, CT], F32)
            nc.sync.dma_start(out=c0s, in_=c0t)
            bo2s = cp.tile([PART, CT], F32)
            nc.sync.dma_start(out=bo2s, in_=bo2t)
            gm = cp.tile([PART, 8], F32R)
            nc.sync.dma_start(out=gm, in_=gmat)
            gmt = cp.tile([8, PART], F32R)
            nc.sync.dma_start(out=gmt, in_=gmatt)
            ones = cp.tile([PART, PART], BT)
            nc.sync.dma_start(out=ones, in_=onesd)
            epst = cp.tile([PART, 1], F32)
            nc.vector.memset(epst, EPS)

            mts = wp.tile([PART, CT, C], BT)
            wvts = wp.tile([PART, CT, C], BT)
            wots = wp.tile([PART, CT, C], BT)
            for j in range(CT):
                nc.sync.dma_start(out=mts[:, j, :], in_=mt[j * PART:(j + 1) * PART, :])
                nc.sync.dma_start(out=wvts[:, j, :], in_=wvt[j * PART:(j + 1) * PART, :])
                nc.sync.dma_start(out=wots[:, j, :], in_=wot[j * PART:(j + 1) * PART, :])

            hts = [hp.tile([PART, N], BT, tag=f"h{ci}", name=f"h{ci}") for ci in range(CT)]

            # ---- Phase A+B: GroupNorm stats + normalize into h ----
            with (
                tc.tile_pool(name="stats", bufs=2) as sp,
                tc.tile_pool(name="coef", bufs=1) as cfp,
                tc.tile_pool(name="xst", bufs=2) as xp,
                tc.tile_pool(name="pst", bufs=2, space="PSUM") as pp,
            ):
                acoef = cfp.tile([PART, CT], F32)
                bcoef = cfp.tile([PART, CT], F32)
                for ci in range(CT):
                    xt = xp.tile([PART, N], F32, tag="x")
                    nc.sync.dma_start(out=xt, in_=xa[ci * PART:(ci + 1) * PART, :])
                    xr = xt.rearrange("p (s f) -> p s f", f=512)
                    st6 = sp.tile([PART, 8, 6], F32, tag="st6")
                    for s in range(8):
                        nc.vector.bn_stats(out=st6[:, s, :], in_=xr[:, s, :])
                    mv = sp.tile([PART, 2], F32, tag="mv")
                    nc.vector.bn_aggr(out=mv, in_=st6)
                    # s2 = (mean, E[x^2]) per channel
                    s2 = sp.tile([PART, 2], F32R, tag="s2")
                    nc.vector.tensor_copy(out=s2[:, 0:1], in_=mv[:, 0:1])
                    nc.vector.tensor_tensor(
                        out=s2[:, 1:2], in0=mv[:, 0:1], in1=mv[:, 0:1], op=OP.mult)
                    nc.vector.tensor_add(out=s2[:, 1:2], in0=s2[:, 1:2], in1=mv[:, 1:2])
                    # group means over 16-channel blocks: [8, 2]
                    gp_ = pp.tile([8, 2], F32, tag="gp")
                    nc.tensor.matmul(gp_, lhsT=gm, rhs=s2,
                                     start=True, stop=True)
                    gs = sp.tile([8, 2], F32R, tag="gs")
                    nc.vector.tensor_copy(out=gs, in_=gp_)
                    msq = sp.tile([8, 1], F32, tag="msq")
                    nc.vector.tensor_tensor(
                        out=msq, in0=gs[:, 0:1], in1=gs[:, 0:1], op=OP.mult)
                    nc.vector.tensor_sub(out=gs[:, 1:2], in0=gs[:, 1:2], in1=msq)
                    nc.scalar.activation(out=gs[:, 1:2], in_=gs[:, 1:2],
                                         func=AF.Sqrt, bias=epst[0:8])
                    with nc.allow_low_precision(
                            reason="fp32r rounding for PE broadcast matmul"):
                        nc.vector.reciprocal(out=gs[:, 1:2], in_=gs[:, 1:2])
                    # broadcast per-group (mean, rstd) back to channels
                    cb = pp.tile([PART, 2], F32, tag="cb")
                    nc.tensor.matmul(cb, lhsT=gmt, rhs=gs,
                                     start=True, stop=True)
                    nc.vector.tensor_tensor(
                        out=acoef[:, ci:ci + 1], in0=cb[:, 1:2],
                        in1=gwt[:, ci:ci + 1], op=OP.mult)
                    tmpb = sp.tile([PART, 1], F32, tag="tmpb")
                    nc.vector.tensor_tensor(
                        out=tmpb, in0=cb[:, 0:1], in1=acoef[:, ci:ci + 1], op=OP.mult)
                    nc.vector.tensor_sub(
                        out=bcoef[:, ci:ci + 1], in0=gbt[:, ci:ci + 1], in1=tmpb)
                    # h = x * a + b
                    nc.vector.tensor_scalar(
                        out=hts[ci], in0=xt,
                        scalar1=acoef[:, ci:ci + 1], scalar2=bcoef[:, ci:ci + 1],
                        op0=OP.mult, op1=OP.add)

            # ---- Phase C: v^T tiles ----
            with tc.tile_pool(name="vtp", bufs=1) as vp:
                vts = vp.tile([PART, NKT, CH], BT)
                with tc.tile_pool(name="vps", bufs=2, space="PSUM") as vpp:
                    for t in range(NKT):
                        vps = vpp.tile([PART, CH], F32, tag="vps")
                        for ci in range(CT):
                            nc.tensor.matmul(
                                vps,
                                lhsT=hts[ci][:, t * PART:(t + 1) * PART],
                                rhs=wvts[:, ci, :],
                                start=(ci == 0), stop=(ci == CT - 1))
                        nc.vector.tensor_copy(out=vts[:, t, :], in_=vps)

                # ---- Phase D+E: attention + out-projection, per nq-chunk ----
                with (
                    tc.tile_pool(name="ujp", bufs=2) as up,
                    tc.tile_pool(name="ep", bufs=3) as ep,
                    tc.tile_pool(name="attp", bufs=1) as ap_,
                    tc.tile_pool(name="rcp", bufs=2) as rp,
                    tc.tile_pool(name="xrp", bufs=2) as xrp,
                    tc.tile_pool(name="otp", bufs=2) as otp,
                    tc.tile_pool(name="oup", bufs=1, space="PSUM") as oup,
                    tc.tile_pool(name="stp", bufs=2, space="PSUM") as stp,
                    tc.tile_pool(name="ddp", bufs=1, space="PSUM") as ddp,
                    tc.tile_pool(name="upp", bufs=1, space="PSUM") as upp,
                ):
                    def compute_u(jc):
                        # u_jc = M h[:, chunk jc] + c0
                        ut = up.tile([PART, CT, CH], BT, tag="uj", name=f"uj{jc}")
                        sl = slice(jc * CH, (jc + 1) * CH)
                        for i in range(CT):
                            ups = upp.tile([PART, CH], F32, tag="up")
                            for jj in range(CT):
                                nc.tensor.matmul(
                                    ups,
                                    lhsT=mts[:, jj, i * PART:(i + 1) * PART],
                                    rhs=hts[jj][:, sl],
                                    start=(jj == 0), stop=(jj == CT - 1))
                            nc.vector.tensor_scalar(
                                out=ut[:, i, :], in0=ups,
                                scalar1=c0s[:, i:i + 1], scalar2=None, op0=OP.add)
                        return ut

                    uj_next = compute_u(0)
                    for j in range(JCH):
                        jsl = slice(j * CH, (j + 1) * CH)
                        uj = uj_next
                        # attention accumulation over key tiles
                        ou = [oup.tile([PART, CH], F32, tag=f"ou{m}", name=f"ou{m}_{j}") for m in range(CT)]
                        dd = ddp.tile([PART, CH], F32, tag="dd")
                        for t in range(NKT):
                            st = stp.tile([PART, CH], F32, tag="st")
                            for ci in range(CT):
                                nc.tensor.matmul(
                                    st,
                                    lhsT=hts[ci][:, t * PART:(t + 1) * PART],
                                    rhs=uj[:, ci, :],
                                    start=(ci == 0), stop=(ci == CT - 1))
                            et = ep.tile([PART, CH], BT, tag="et")
                            nc.scalar.activation(out=et, in_=st, func=AF.Exp, scale=SCALE)
                            etr = et[:]
                            for m in range(CT):
                                nc.tensor.matmul(
                                    ou[m],
                                    lhsT=vts[:, t, m * PART:(m + 1) * PART],
                                    rhs=etr,
                                    start=(t == 0), stop=(t == NKT - 1))
                            nc.tensor.matmul(
                                dd, lhsT=ones, rhs=etr,
                                start=(t == 0), stop=(t == NKT - 1))
                            if t == NKT - 3 and j + 1 < JCH:
                                uj_next = compute_u(j + 1)
                        # normalize
                        rc = rp.tile([PART, CH], F32, tag="rc")
                        nc.vector.reciprocal(out=rc, in_=dd)
                        att = ap_.tile([PART, CT, CH], BT, tag="att")
                        for m in range(CT):
                            nc.vector.tensor_mul(out=att[:, m, :], in0=ou[m], in1=rc)
                        # out-projection + bias + residual
                        for m in range(CT):
                            fp = oup.tile([PART, CH], F32, tag=f"ou{m}")
                            for ci in range(CT):
                                nc.tensor.matmul(
                                    fp,
                                    lhsT=wots[:, ci, m * PART:(m + 1) * PART],
                                    rhs=att[:, ci, :],
                                    start=(ci == 0), stop=(ci == CT - 1))
                            xr_ = xrp.tile([PART, CH], F32, tag="xr")
                            nc.sync.dma_start(
                                out=xr_, in_=xa[m * PART:(m + 1) * PART, jsl])
                            ot = otp.tile([PART, CH], F32, tag="ot")
                            nc.vector.tensor_scalar(
                                out=ot, in0=fp,
                                scalar1=bo2s[:, m:m + 1], scalar2=None, op0=OP.add)
                            nc.vector.tensor_add(out=ot, in0=ot, in1=xr_)
                            nc.sync.dma_start(
                                out=y[m * PART:(m + 1) * PART, jsl], in_=ot)
    if split_waits:
        split_multi_waits(nc)
    return nc


def prep_inputs(x, gn_w, gn_b, qkv_w, qkv_b, out_w, out_b):
    x = np.asarray(x, np.float32)
    gn_w = np.asarray(gn_w, np.float32)
    gn_b = np.asarray(gn_b, np.float32)
    qkv_w = np.asarray(qkv_w, np.float32)
    qkv_b = np.asarray(qkv_b, np.float32)
    out_w = np.asarray(out_w, np.float32)
    out_b = np.asarray(out_b, np.float32)

    Wq, Wk, Wv = qkv_w[0:C], qkv_w[C:2 * C], qkv_w[2 * C:3 * C]
    bq, bv = qkv_b[0:C], qkv_b[2 * C:3 * C]
    bf16 = ml_dtypes.bfloat16
    mt = np.ascontiguousarray((Wq.T @ Wk).astype(bf16))          # (M = Wk^T Wq).T
    wvt = np.ascontiguousarray(Wv.T.astype(bf16))
    wot = np.ascontiguousarray(out_w.T.astype(bf16))
    c0 = (Wk.T @ bq).astype(np.float32)
    bo2 = (out_w @ bv + out_b).astype(np.float32)

    def coltiles(v):
        return np.ascontiguousarray(v.reshape(CT, PART).T, dtype=np.float32)

    gmat = np.zeros((PART, 8), np.float32)
    gmatt = np.zeros((8, PART), np.float32)
    for p in range(PART):
        gmat[p, p // 16] = 1.0 / 16.0
        gmatt[p // 16, p] = 1.0
    shared = {
        "mt": mt, "wvt": wvt, "wot": wot,
        "gw": coltiles(gn_w), "gb": coltiles(gn_b),
        "c0t": coltiles(c0), "bo2t": coltiles(bo2),
        "gmat": gmat, "gmatt": gmatt,
        "onesd": np.ones((PART, PART), ml_dtypes.bfloat16),
    }
    in_maps = []
    for core in range(8):
        br, hf = divmod(core, 2)
        xa = x[br].reshape(C, N)
        if hf:
            xa = np.concatenate([xa[:, NQ:], xa[:, :NQ]], axis=1)
        xa = np.ascontiguousarray(xa, dtype=np.float32)
        in_maps.append({"xa": xa, **shared})
    return in_maps


def assemble_output(results, b=4, hh=64, ww=64):
    out = np.zeros((b, C, N), np.float32)
    for core in range(8):
        br, hf = divmod(core, 2)
        out[br][:, hf * NQ:(hf + 1) * NQ] = results[core]["y"]
    return out.reshape(b, C, hh, ww)


def kernel(x, gn_w, gn_b, qkv_w, qkv_b, out_w, out_b):
    from concourse import bass_utils
    in_maps = prep_inputs(x, gn_w, gn_b, qkv_w, qkv_b, out_w, out_b)
    nc = build_program()
    res = bass_utils.run_bass_kernel_spmd(nc, in_maps, core_ids=list(range(8)))
    return assemble_output(res.results)
